# revision 1
# baseline (speedup 1.0000x reference)
"""Trainium2 Bass kernel: correlation (cost volume) layer.

kernel(in1, in2): full inputs [8, 256, 96, 192] f32 -> output [8, 25, 96, 192] f32.
Sharding: data-parallel over batch, one batch per NeuronCore (8 cores, SPMD).
"""
import sys
if '/opt/trn_rl_repo' not in sys.path:
    sys.path.insert(0, '/opt/trn_rl_repo')
import numpy as np

"""Correlation (cost volume) kernel for Trainium2 — verifier-legal AP edition.

out[d=(a,b), h, w] = mean_c in1[c,h,w] * in2pad[c, h+2a-4, w+2b-4],  a,b in 0..4

Design notes (walrus requires matmul stationary AND moving APs to each have
exactly ONE free dim):
- in1 parity tiles are pre-packed contiguous ([128c, 128m], m = j*8+i).
- in2 parity planes are pre-packed COLUMN-MAJOR ([128c, 100 s, 12 r]); a
  [12 x 20] halo window then spans all 12 rows of the strip-plane, so it is a
  single contiguous 240-element run -> one matmul per K-chunk, rhs AP [[1,240]].
- psum[m, n=s*12+r] band: needed entries at n = n_m + (12b + a), n_m = 12j + i.
  Sheared through DRAM scratch (per-j-block DMAs; j-block = 8 partitions with
  uniform column ranges), read back aligned, compacted 53->25 by a strided
  copy, transposed by TensorE (contiguous [[1,25]] stationary AP).
"""

import concourse.bass as bass
import concourse.mybir as mybir

f32 = mybir.dt.float32

TH, TW = 8, 16            # parity tile shape; partition m = j*8 + i
WH, WW = TH + 4, TW + 4   # window 12 x 20
NW = WH * WW              # 240
BAND = 4 * WH + 4 + 1     # 53 = span of {12b + a}
NCOLS = BAND + TH - 1     # 60 columns written per row in the shear
SHIFT0 = TH - 1           # 7
PITCH = 72                # scratch row pitch (>= SHIFT0 + NCOLS = 67)
PARITIES = ((0, 0), (0, 1), (1, 0), (1, 1))


def build_corr(nc, tc, in1_d, in2_d, out_d, scratch_d, C, H, W):
    from concourse import masks

    HP, WP = H // 2, W // 2
    NJ = WP // TW
    NSTRIP = HP // TH
    HW = H * W
    KC = C // 128
    IN2R = 2 * TH + 8             # 24 natural strip rows incl +-4 halo
    IN2C = W + 8                  # 200 natural cols incl +-4 halo
    PKC = WP + 4                  # 100 packed plane cols (+-2 parity halo)
    TILE_SCR = PITCH * 128
    NT = NJ
    NTA = (NT + 1) // 2
    inv_c = 1.0 / C

    with (
        tc.tile_pool(name="const", bufs=1) as cpool,
        tc.tile_pool(name="in1p", bufs=2) as in1_pool,
        tc.tile_pool(name="pk1", bufs=2) as pk1_pool,
        tc.tile_pool(name="pk2", bufs=2) as pk2_pool,
        tc.tile_pool(name="spool", bufs=2) as s_pool,
        tc.tile_pool(name="upool", bufs=2) as u_pool,
        tc.tile_pool(name="ypool", bufs=2) as y_pool,
        tc.tile_pool(name="opool", bufs=2) as o_pool,
        tc.tile_pool(name="psumw", bufs=4, space="PSUM") as pw_pool,
        tc.tile_pool(name="psum2", bufs=4, space="PSUM") as p2_pool,
    ):
        identity = cpool.tile([128, 128], f32)
        masks.make_identity(nc, identity[:])

        in2_bufs = []
        for ib in range(2):
            in2_buf = cpool.tile([128, KC, IN2R, IN2C], f32, tag=f"in2_{ib}")
            in2_bufs.append(in2_buf)
        for b in in2_bufs:
            nc.vector.memset(b[:, :, :, 0:4], 0.0)
            nc.vector.memset(b[:, :, :, IN2C - 4:IN2C], 0.0)

        for s in range(NSTRIP):
            # ---- load strips ----
            in1_s = in1_pool.tile([128, KC, 2 * TH, W], f32)
            for k in range(KC):
                nc.sync.dma_start(
                    in1_s[:, k],
                    bass.AP(in1_d, k * 128 * HW + s * 2 * TH * W,
                            [[HW, 128], [W, 2 * TH], [1, W]]))
            in2_s = in2_bufs[s % 2]
            r_lo = s * 2 * TH - 4
            v_lo, v_hi = max(r_lo, 0), min(r_lo + IN2R, H)
            nrow = v_hi - v_lo
            if r_lo < 0:
                nc.vector.memset(in2_s[:, :, 0:(v_lo - r_lo), 4:4 + W], 0.0)
            if r_lo + IN2R > H:
                nc.vector.memset(
                    in2_s[:, :, IN2R - (r_lo + IN2R - H):IN2R, 4:4 + W], 0.0)
            for k in range(KC):
                nc.sync.dma_start(
                    in2_s[:, k, (v_lo - r_lo):(v_lo - r_lo) + nrow, 4:4 + W],
                    bass.AP(in2_d, k * 128 * HW + v_lo * W,
                            [[HW, 128], [W, nrow], [1, W]]))

            o_sbuf = o_pool.tile([25, 2 * TH, W], f32)
            in1_ap = in1_s[:]
            in2_ap = in2_s[:]
            p_in1 = in1_ap.ap[0][0]
            p_in2 = in2_ap.ap[0][0]

            for pi, (py, px) in enumerate(PARITIES):
                # ---- pack in1 tiles contiguous: pk1[c, k, t, m], m = j*8+i ----
                pk1 = pk1_pool.tile([128, KC, NT, 128], f32, tag="pk1")
                for k in range(KC):
                    src = bass.AP(
                        in1_ap.tensor,
                        in1_ap.offset + k * 2 * TH * W + py * W + px,
                        [[p_in1, 128], [2 * TW, NT], [2, TW], [2 * W, TH]])
                    eng = nc.scalar if (k + pi) % 2 == 0 else nc.vector
                    if eng is nc.scalar:
                        nc.scalar.copy(pk1[:, k], src)
                    else:
                        nc.vector.tensor_copy(pk1[:, k], src)

                # ---- pack in2 plane column-major: pk2[c, k, s', r] ----
                pk2 = pk2_pool.tile([128, KC, PKC, WH], f32, tag="pk2")
                for k in range(KC):
                    src = bass.AP(
                        in2_ap.tensor,
                        in2_ap.offset + k * IN2R * IN2C + py * IN2C + px,
                        [[p_in2, 128], [2, PKC], [2 * IN2C, WH]])
                    eng_sc = (k + pi) % 2 == 1
                    if eng_sc:
                        nc.scalar.copy(pk2[:, k], src)
                    else:
                        nc.vector.tensor_copy(pk2[:, k], src)

                # ---- window matmuls ----
                S = s_pool.tile([128, NT, NW], f32)
                pk2_ap = pk2[:]
                p_pk2 = pk2_ap.ap[0][0]
                for t in range(NT):
                    pw = pw_pool.tile([128, NW], f32)
                    for k in range(KC):
                        rhs = bass.AP(
                            pk2_ap.tensor,
                            pk2_ap.offset + k * PKC * WH + t * TW * WH,
                            [[p_pk2, 128], [1, NW]])
                        nc.tensor.matmul(
                            pw[:], pk1[:, k, t, :], rhs,
                            start=(k == 0), stop=(k == KC - 1))
                    if t % 2 == 0:
                        nc.scalar.copy(S[:, t, :], pw[:])
                    else:
                        nc.vector.tensor_copy(S[:, t, :], pw[:])

                # ---- sheared write: one DMA per j-block of 8 partitions ----
                # m = j*8 + i; n_m = 12j + i; write cols [12j, 12j + NCOLS)
                # scratch[base + m*PITCH + SHIFT0 + (col - n_m)]
                sg_base = (s % 3) * 4 + pi
                scr_base = sg_base * NT * TILE_SCR
                s_ap = S[:]
                p_s = s_ap.ap[0][0]
                for j in range(TW):
                    sap = bass.AP(s_ap.tensor,
                                  s_ap.offset + 8 * j * p_s + WH * j,
                                  [[p_s, 8], [NW, NT], [1, NCOLS]])
                    dap = bass.AP(scratch_d,
                                  scr_base + 8 * j * PITCH + SHIFT0,
                                  [[PITCH - 1, 8], [TILE_SCR, NT], [1, NCOLS]])
                    nc.sync.dma_start(dap, sap)

                # ---- aligned read-back ----
                U = u_pool.tile([128, NT, BAND], f32)
                nc.sync.dma_start(
                    U[:],
                    bass.AP(scratch_d, scr_base + SHIFT0,
                            [[PITCH, 128], [TILE_SCR, NT], [1, BAND]]))

                # ---- compact 53 -> 25: Y[m, t, d=(a,b)] = U[m, t, 12b+a] ----
                Y = y_pool.tile([128, NT, 25], f32)
                u_ap = U[:]
                src = bass.AP(u_ap.tensor, u_ap.offset,
                              [[u_ap.ap[0][0], 128], [BAND, NT], [1, 5], [WH, 5]])
                if pi % 2 == 0:
                    nc.scalar.copy(Y[:], src)
                else:
                    nc.vector.tensor_copy(Y[:], src)

                # ---- transpose: psum2[d, m] per tile ----
                p2s = []
                for g in range(2):
                    gnt = min(NTA, NT - g * NTA)
                    p2 = p2_pool.tile([25, NTA * 128], f32, tag="p2")
                    p2s.append(p2)
                    for tt in range(gnt):
                        t = g * NTA + tt
                        nc.tensor.transpose(
                            p2[:, tt * 128:(tt + 1) * 128], Y[:, t, :],
                            identity[:])

                # ---- assemble with 1/C scaling; m=(j,i) -> h=2i+py, w=2(16t+j)+px
                o_ap = o_sbuf[:]
                p_o = o_ap.ap[0][0]
                for g in range(2):
                    gnt = min(NTA, NT - g * NTA)
                    p2_ap = p2s[g][:]
                    src = bass.AP(p2_ap.tensor, p2_ap.offset,
                                  [[p2_ap.ap[0][0], 25], [128, gnt],
                                   [TH, TW], [1, TH]])
                    dst = bass.AP(o_ap.tensor,
                                  o_ap.offset + py * W + px + g * NTA * 2 * TW,
                                  [[p_o, 25], [2 * TW, gnt], [2, TW],
                                   [2 * W, TH]])
                    if pi % 2 == 0:
                        nc.scalar.mul(dst, src, inv_c)
                    else:
                        nc.vector.tensor_scalar_mul(dst, src, inv_c)

            nc.sync.dma_start(
                bass.AP(out_d, s * 2 * TH * W, [[HW, 25], [W, 2 * TH], [1, W]]),
                o_sbuf[:])


def scratch_elems(W):
    return 12 * (W // 2 // TW) * PITCH * 128


def build_module(C=256, H=96, W=192):
    import concourse.bacc as bacc
    import concourse.tile as tile
    nc = bacc.Bacc("TRN2", target_bir_lowering=False, debug=False)
    in1_d = nc.dram_tensor("in1", [C, H, W], f32, kind="ExternalInput")
    in2_d = nc.dram_tensor("in2", [C, H, W], f32, kind="ExternalInput")
    out_d = nc.dram_tensor("out", [25, H, W], f32, kind="ExternalOutput")
    scratch_d = nc.dram_tensor("scratch", [scratch_elems(W)], f32)
    with tile.TileContext(nc) as tc:
        build_corr(nc, tc, in1_d, in2_d, out_d, scratch_d, C, H, W)
    nc.compile()
    return nc


def reference_np(in1, in2, md=4, st=2):
    import numpy as np
    in1, in2 = in1[None], in2[None]
    B, C, H, W = in1.shape
    in2p = np.pad(in2, ((0, 0), (0, 0), (md, md), (md, md)))
    outs = []
    for dy in range(0, 2 * md + 1, st):
        for dx in range(0, 2 * md + 1, st):
            outs.append((in1 * in2p[:, :, dy:dy + H, dx:dx + W]).mean(axis=1))
    return np.stack(outs, axis=1)[0]


B_FULL, C_FULL, H_FULL, W_FULL = 8, 256, 96, 192
_NC = None


def _get_nc():
    global _NC
    if _NC is None:
        _NC = build_module(C_FULL, H_FULL, W_FULL)
    return _NC


def kernel(in1, in2):
    from concourse.bass_utils import run_bass_kernel_spmd
    in1 = np.ascontiguousarray(np.asarray(in1, dtype=np.float32))
    in2 = np.ascontiguousarray(np.asarray(in2, dtype=np.float32))
    assert in1.shape == (B_FULL, C_FULL, H_FULL, W_FULL), in1.shape
    nc = _get_nc()
    in_maps = [{"in1": in1[b], "in2": in2[b]} for b in range(B_FULL)]
    res = run_bass_kernel_spmd(nc, in_maps, core_ids=list(range(B_FULL)))
    out = np.stack(
        [np.asarray(res.results[b]["out"]).reshape(25, H_FULL, W_FULL)
         for b in range(B_FULL)], axis=0)
    return out



# revision 2
# speedup vs baseline: 1.0998x; 1.0998x over previous
"""Trainium2 Bass kernel v2: correlation (cost volume) layer.

kernel(in1, in2): full inputs [8, 256, 96, 192] f32 -> output [8, 25, 96, 192] f32.
Sharding: data-parallel over batch, one batch per NeuronCore (8 cores, SPMD).

out[d=(a,b), h, w] = mean_c in1[c,h,w] * in2pad[c, h+2a-4, w+2b-4],  a,b in 0..4

v2 design vs baseline:
- fp16 matmul inputs (1 cyc/row vs fp32's 4; FWL weight-load overlap).
- Tall tiles: TH=16 rows x TW=8 cols on the parity grid -> 8 j-blocks
  (16-partition shear DMAs) instead of 16, 3 strips of 32 rows instead
  of 6 of 16.
- Band extraction via DRAM shear scratch in fp16 with tile-packed
  columns: per (partition, parity) the 12 tiles' 100-col windows abut
  -> one contiguous 2400B descriptor run per partition (full DMA bw),
  and the whole readback is one DMA per parity-pair.
- Shear fused over (tile, parity-pair): 8 shear DMAs + 1 readback per
  pair; ~70 DMAs total vs ~440 (DMA issue costs ~0.6us sequencer each).
- in1/in2 loaded once in chunks (no halo re-read); pk2 packs read
  across chunk boundaries.
- Pack work split across scalar/vector/gpsimd engines.
- Emission software-pipelined: strip s's post-processing (compact,
  transposes, assemble) is emitted inside strip s+1 so the PE never
  queues dep-blocked transposes ahead of ready matmuls.
"""
import sys
if '/opt/trn_rl_repo' not in sys.path:
    sys.path.insert(0, '/opt/trn_rl_repo')
import numpy as np

import concourse.bass as bass
import concourse.mybir as mybir

f32 = mybir.dt.float32
f16 = mybir.dt.float16

C_FULL, H_FULL, W_FULL = 256, 96, 192
B_FULL = 8

TH, TW = 16, 8            # parity-grid tile: m = 16*j + i
WH, WW = TH + 4, TW + 4   # window 20 rows x 12 col-strips
NW = WH * WW              # 240
BAND = 4 * WH + 4 + 1     # 85 = span of {20b + a}
NCOLS = BAND + TH - 1     # 100 columns written per (row, tile)
SHIFT0 = TH - 1           # 15
PITCH = SHIFT0 + 12 * NCOLS + 1   # 1216 per-partition scratch row (f16 elems)
PARITIES = ((0, 0), (0, 1), (1, 0), (1, 1))  # (py, px); pairs share py
IN2CH = 24                # in2 chunk rows
IN1CH = 16                # in1 chunk rows


def build_corr(nc, tc, in1_d, in2_d, out_d, scratch_d, C, H, W):
    from concourse import masks

    KC = C // 128
    HW = H * W
    NT = (W // 2) // TW       # 12 tiles per parity row-strip
    SROWS = 2 * TH            # 32 natural rows per strip
    NSTRIP = H // SROWS       # 3
    PKC = (W // 2) + 4        # 100 packed plane cols (+-2 parity halo)
    RPAR = 128 * PITCH        # scratch region per parity
    ULEN = (NT - 1) * NCOLS + BAND  # 1185 readback cols per parity
    NCH2 = H // IN2CH + 1     # 4 in2 chunks (last partial-use)
    NCH1 = H // IN1CH         # 6 in1 chunks
    inv_c = 1.0 / C

    with (
        tc.tile_pool(name="const", bufs=1) as cpool,
        tc.tile_pool(name="in2c", bufs=2) as in2_pool,
        tc.tile_pool(name="in1c", bufs=2) as in1_pool,
        tc.tile_pool(name="pk1", bufs=4) as pk1_pool,
        tc.tile_pool(name="pk2", bufs=2) as pk2_pool,
        tc.tile_pool(name="spool", bufs=1) as s_pool,
        tc.tile_pool(name="upool", bufs=2) as u_pool,
        tc.tile_pool(name="ypool", bufs=2) as y_pool,
        tc.tile_pool(name="opool", bufs=1) as o_pool,
        tc.tile_pool(name="psumw", bufs=4, space="PSUM") as pw_pool,
        tc.tile_pool(name="psum2", bufs=2, space="PSUM") as p2_pool,
    ):
        identity = cpool.tile([128, 128], f16)
        masks.make_identity(nc, identity[:])

        chunk_in2 = {}
        chunk_in1 = {}

        def load_in2_chunk(c):
            t_ = in2_pool.tile([128, KC, IN2CH, W], f32, tag="in2c",
                               name="in2chunk")
            chunk_in2[c] = t_
            nc.sync.dma_start(
                t_[:],
                bass.AP(in2_d, c * IN2CH * W,
                        [[HW, 128], [128 * HW, KC], [W, IN2CH], [1, W]]))

        def load_in1_chunk(u):
            t_ = in1_pool.tile([128, KC, IN1CH, W], f32, tag="in1c",
                               name="in1chunk")
            chunk_in1[u] = t_
            nc.sync.dma_start(
                t_[:],
                bass.AP(in1_d, u * IN1CH * W,
                        [[HW, 128], [128 * HW, KC], [W, IN1CH], [1, W]]))

        def pk2_ranges(s, py):
            # (chunk_idx, start_row_in_chunk, r0, nr) covering r in [0,20)
            # natural row of r: 32s - 4 + py + 2r; chunk c = rows [24c, 24c+24)
            if s == 0:
                return [(0, py, 2, 12), (1, py, 14, 6)]
            if s == 1:
                return [(1, 4 + py, 0, 10), (2, py, 10, 10)]
            return [(2, 12 + py, 0, 6), (3, py, 6, 12)]

        def pack_pk1(s):
            pk1_t = {}
            for pi, (py, px) in enumerate(PARITIES):
                pk1_ = pk1_pool.tile([128, KC, NT, 128], f16, tag="pk1",
                                     name="pk1")
                pk1_t[pi] = pk1_
                dap = pk1_[:]
                p_d = dap.ap[0][0]
                for k in range(KC):
                    for ih in range(2):
                        cap = chunk_in1[2 * s + ih][:]
                        src = bass.AP(
                            cap.tensor,
                            cap.offset + k * IN1CH * W + py * W + px,
                            [[cap.ap[0][0], 128], [2 * TW, NT], [2, TW],
                             [2 * W, TH // 2]])
                        dst = bass.AP(
                            dap.tensor,
                            dap.offset + k * NT * 128 + ih * (TH // 2),
                            [[p_d, 128], [128, NT], [TH, TW], [1, TH // 2]])
                        nc.scalar.copy(dst, src)
            return pk1_t

        def pack_pk2(s, pi):
            py, px = PARITIES[pi]
            pk2_ = pk2_pool.tile([128, KC, PKC, WH], f16, tag="pk2",
                                 name="pk2")
            dap = pk2_[:]
            p_d = dap.ap[0][0]
            # zero the +-2 parity col halo (s' in [0,2) and [98,100))
            nc.gpsimd.memset(pk2_[:, :, 0:2, :], 0.0)
            nc.gpsimd.memset(pk2_[:, :, PKC - 2:PKC, :], 0.0)
            # zero rows beyond the image (top of strip 0, bottom of last)
            if s == 0:
                nc.gpsimd.memset(pk2_[:, :, :, 0:2], 0.0)
            if s == NSTRIP - 1:
                nc.gpsimd.memset(pk2_[:, :, :, WH - 2:WH], 0.0)
            for k in range(KC):
                for (ci, row0, r0, nr) in pk2_ranges(s, py):
                    cap = chunk_in2[ci][:]
                    src = bass.AP(
                        cap.tensor,
                        cap.offset + k * IN2CH * W + row0 * W + px,
                        [[cap.ap[0][0], 128], [2, PKC - 4], [2 * W, nr]])
                    dst = bass.AP(
                        dap.tensor,
                        dap.offset + k * PKC * WH + 2 * WH + r0,
                        [[p_d, 128], [WH, PKC - 4], [1, nr]])
                    nc.gpsimd.tensor_copy(dst, src)
            return pk2_

        def matmuls(pk1_, pk2_, S_, pip):
            pk2_ap = pk2_[:]
            p_pk2 = pk2_ap.ap[0][0]
            for t in range(NT):
                pw = pw_pool.tile([128, NW], f32, tag="pw", name="pw")
                for k in range(KC):
                    rhs = bass.AP(pk2_ap.tensor,
                                  pk2_ap.offset + k * PKC * WH + t * TW * WH,
                                  [[p_pk2, 128], [1, NW]])
                    nc.tensor.matmul(pw[:], pk1_[:, k, t, :], rhs,
                                     start=(k == 0), stop=(k == KC - 1))
                if t % 2 == 0:
                    nc.vector.tensor_copy(S_[:, pip, t, :], pw[:])
                else:
                    nc.scalar.copy(S_[:, pip, t, :], pw[:])

        def shear(s, P, S_):
            pair_base = (s % 2) * (4 * RPAR) + 2 * P * RPAR
            s_ap = S_[:]
            p_S = s_ap.ap[0][0]
            for pip in range(2):
                for j in range(TW):
                    sap = bass.AP(s_ap.tensor,
                                  s_ap.offset + 16 * j * p_S + pip * NT * NW
                                  + WH * j,
                                  [[p_S, 16], [NW, NT], [1, NCOLS]])
                    dap = bass.AP(scratch_d,
                                  pair_base + pip * RPAR + 16 * j * PITCH
                                  + SHIFT0,
                                  [[PITCH - 1, 16], [NCOLS, NT], [1, NCOLS]])
                    nc.sync.dma_start(dap, sap)
            U_ = u_pool.tile([128, 2, ULEN], f16, tag="U", name="U")
            nc.sync.dma_start(
                U_[:],
                bass.AP(scratch_d, pair_base + SHIFT0,
                        [[PITCH, 128], [RPAR, 2], [1, ULEN]]))
            return U_

        o_state = {}

        def post(s, P, U_):
            # compact 1185 -> 25, transpose [m,d]->[d,m], assemble to o_sbuf
            if P == 0:
                o_state[s] = o_pool.tile([25, SROWS, W], f32, tag="o",
                                         name="o_sbuf")
            o_sbuf = o_state[s]
            o_ap = o_sbuf[:]
            p_o = o_ap.ap[0][0]
            u_ap = U_[:]
            p_u = u_ap.ap[0][0]
            for pip in range(2):
                pi = 2 * P + pip
                py, px = PARITIES[pi]
                Y_ = y_pool.tile([128, NT, 25], f16, tag="Y", name="Y")
                src = bass.AP(u_ap.tensor, u_ap.offset + pip * ULEN,
                              [[p_u, 128], [NCOLS, NT], [1, 5], [WH, 5]])
                nc.vector.tensor_copy(Y_[:], src)
                p2 = p2_pool.tile([25, NT * 128], f16, tag="p2", name="p2")
                for t in range(NT):
                    nc.tensor.transpose(p2[:, t * 128:(t + 1) * 128],
                                        Y_[:, t, :], identity[:])
                p2_ap = p2[:]
                asrc = bass.AP(p2_ap.tensor, p2_ap.offset,
                               [[p2_ap.ap[0][0], 25], [128, NT], [TH, TW],
                                [1, TH]])
                adst = bass.AP(o_ap.tensor, o_ap.offset + py * W + px,
                               [[p_o, 25], [2 * TW, NT], [2, TW], [2 * W, TH]])
                if pi % 2 == 0:
                    nc.scalar.mul(adst, asrc, inv_c)
                else:
                    nc.vector.tensor_scalar_mul(adst, asrc, inv_c)

        def out_dma(s):
            nc.sync.dma_start(
                bass.AP(out_d, s * SROWS * W, [[HW, 25], [W, SROWS], [1, W]]),
                o_state[s][:])

        # prologue loads
        load_in2_chunk(0)
        load_in2_chunk(1)
        load_in1_chunk(0)
        load_in1_chunk(1)

        U_hist = {}
        for s in range(NSTRIP):
            pk1_t = pack_pk1(s)
            if s > 0:
                post(s - 1, 0, U_hist[(s - 1, 0)])
            if s + 1 < NSTRIP:
                load_in1_chunk(2 * s + 2)
                load_in1_chunk(2 * s + 3)
            for P in range(2):
                S_ = s_pool.tile([128, 2, NT, NW], f16, tag="S", name="S")
                for pip in range(2):
                    pi = 2 * P + pip
                    pk2_ = pack_pk2(s, pi)
                    if pi == 3 and s + 1 < NSTRIP:
                        load_in2_chunk(s + 2)
                    matmuls(pk1_t[pi], pk2_, S_, pip)
                U_hist[(s, P)] = shear(s, P, S_)
                if P == 0 and s > 0:
                    post(s - 1, 1, U_hist[(s - 1, 1)])
                    out_dma(s - 1)
        post(NSTRIP - 1, 0, U_hist[(NSTRIP - 1, 0)])
        post(NSTRIP - 1, 1, U_hist[(NSTRIP - 1, 1)])
        out_dma(NSTRIP - 1)


def scratch_elems():
    return 2 * 4 * 128 * PITCH


def build_module(C=256, H=96, W=192):
    import concourse.bacc as bacc
    import concourse.tile as tile
    nc = bacc.Bacc("TRN2", target_bir_lowering=False, debug=False)
    in1_d = nc.dram_tensor("in1", [C, H, W], f32, kind="ExternalInput")
    in2_d = nc.dram_tensor("in2", [C, H, W], f32, kind="ExternalInput")
    out_d = nc.dram_tensor("out", [25, H, W], f32, kind="ExternalOutput")
    scratch_d = nc.dram_tensor("scratch", [scratch_elems()], f16)
    with tile.TileContext(nc) as tc:
        build_corr(nc, tc, in1_d, in2_d, out_d, scratch_d, C, H, W)
    nc.compile()
    return nc


def reference_np(in1, in2, md=4, st=2):
    in1, in2 = in1[None], in2[None]
    B, C, H, W = in1.shape
    in2p = np.pad(in2, ((0, 0), (0, 0), (md, md), (md, md)))
    outs = []
    for dy in range(0, 2 * md + 1, st):
        for dx in range(0, 2 * md + 1, st):
            outs.append((in1 * in2p[:, :, dy:dy + H, dx:dx + W]).mean(axis=1))
    return np.stack(outs, axis=1)[0]


_NC = None


def _get_nc():
    global _NC
    if _NC is None:
        _NC = build_module(C_FULL, H_FULL, W_FULL)
    return _NC


def kernel(in1, in2):
    from concourse.bass_utils import run_bass_kernel_spmd
    in1 = np.ascontiguousarray(np.asarray(in1, dtype=np.float32))
    in2 = np.ascontiguousarray(np.asarray(in2, dtype=np.float32))
    assert in1.shape == (B_FULL, C_FULL, H_FULL, W_FULL), in1.shape
    nc = _get_nc()
    in_maps = [{"in1": in1[b], "in2": in2[b]} for b in range(B_FULL)]
    res = run_bass_kernel_spmd(nc, in_maps, core_ids=list(range(B_FULL)))
    out = np.stack(
        [np.asarray(res.results[b]["out"]).reshape(25, H_FULL, W_FULL)
         for b in range(B_FULL)], axis=0)
    return out


# revision 3
# speedup vs baseline: 1.1649x; 1.0592x over previous
"""Trainium2 Bass kernel v2: correlation (cost volume) layer.

kernel(in1, in2): full inputs [8, 256, 96, 192] f32 -> output [8, 25, 96, 192] f32.
Sharding: data-parallel over batch, one batch per NeuronCore (8 cores, SPMD).

out[d=(a,b), h, w] = mean_c in1[c,h,w] * in2pad[c, h+2a-4, w+2b-4],  a,b in 0..4

v2 design vs baseline:
- fp16 matmul inputs (1 cyc/row vs fp32's 4; FWL weight-load overlap).
- Tall tiles: TH=16 rows x TW=8 cols on the parity grid -> 8 j-blocks
  (16-partition shear DMAs) instead of 16, 3 strips of 32 rows instead
  of 6 of 16.
- Band extraction via DRAM shear scratch in fp16 with tile-packed
  columns: per (partition, parity) the 12 tiles' 100-col windows abut
  -> one contiguous 2400B descriptor run per partition (full DMA bw),
  and the whole readback is one DMA per parity-pair.
- Shear fused over (tile, parity-pair): 8 shear DMAs + 1 readback per
  pair; ~70 DMAs total vs ~440 (DMA issue costs ~0.6us sequencer each).
- in1/in2 loaded once in chunks (no halo re-read); pk2 packs read
  across chunk boundaries.
- Pack work split across scalar/vector/gpsimd engines, weighted by
  measured per-engine strided-copy rates (vector ~1.4 ns/el, scalar
  ~1.7, gpsimd ~3.5).
- Emission software-pipelined: strip s's post-processing (compact,
  transposes, assemble) is emitted inside strip s+1 so the PE never
  queues dep-blocked transposes ahead of ready matmuls.
"""
import sys
if '/opt/trn_rl_repo' not in sys.path:
    sys.path.insert(0, '/opt/trn_rl_repo')
import numpy as np

import concourse.bass as bass
import concourse.mybir as mybir

f32 = mybir.dt.float32
f16 = mybir.dt.float16

C_FULL, H_FULL, W_FULL = 256, 96, 192
B_FULL = 8

TH, TW = 16, 8            # parity-grid tile: m = 16*j + i
WH, WW = TH + 4, TW + 4   # window 20 rows x 12 col-strips
NW = WH * WW              # 240
BAND = 4 * WH + 4 + 1     # 85 = span of {20b + a}
NCOLS = BAND + TH - 1     # 100 columns written per (row, tile)
SHIFT0 = TH - 1           # 15
PITCH = SHIFT0 + 12 * NCOLS + 1   # 1216 per-partition scratch row (f16 elems)
PARITIES = ((0, 0), (0, 1), (1, 0), (1, 1))  # (py, px); pairs share py
IN2CH = 24                # in2 chunk rows
IN1CH = 16                # in1 chunk rows


def build_corr(nc, tc, in1_d, in2_d, out_d, scratch_d, C, H, W):
    from concourse import masks

    KC = C // 128
    HW = H * W
    NT = (W // 2) // TW       # 12 tiles per parity row-strip
    SROWS = 2 * TH            # 32 natural rows per strip
    NSTRIP = H // SROWS       # 3
    PKC = (W // 2) + 4        # 100 packed plane cols (+-2 parity halo)
    RPAR = 128 * PITCH        # scratch region per parity
    ULEN = (NT - 1) * NCOLS + BAND  # 1185 readback cols per parity
    NCH2 = H // IN2CH + 1     # 4 in2 chunks (last partial-use)
    NCH1 = H // IN1CH         # 6 in1 chunks
    inv_c = 1.0 / C

    with (
        tc.tile_pool(name="const", bufs=1) as cpool,
        tc.tile_pool(name="in2c", bufs=2) as in2_pool,
        tc.tile_pool(name="in1c", bufs=2) as in1_pool,
        tc.tile_pool(name="pk1", bufs=4) as pk1_pool,
        tc.tile_pool(name="pk2", bufs=2) as pk2_pool,
        tc.tile_pool(name="spool", bufs=1) as s_pool,
        tc.tile_pool(name="upool", bufs=2) as u_pool,
        tc.tile_pool(name="ypool", bufs=2) as y_pool,
        tc.tile_pool(name="opool", bufs=1) as o_pool,
        tc.tile_pool(name="psumw", bufs=4, space="PSUM") as pw_pool,
        tc.tile_pool(name="psum2", bufs=2, space="PSUM") as p2_pool,
    ):
        identity = cpool.tile([128, 128], f16)
        masks.make_identity(nc, identity[:])

        # weighted engine cycle for pk2 gathers: capacity-proportional
        PK2_CYCLE = ('g', 'v', 'g', 's', 'g', 'v', 'g', 'v')
        pk2_rr = [0]

        def pk2_engine():
            pk2_rr[0] = (pk2_rr[0] + 1) % len(PK2_CYCLE)
            return {'s': nc.scalar, 'v': nc.vector,
                    'g': nc.gpsimd}[PK2_CYCLE[pk2_rr[0]]]

        chunk_in2 = {}
        chunk_in1 = {}

        def load_in2_chunk(c):
            t_ = in2_pool.tile([128, KC, IN2CH, W], f32, tag="in2c",
                               name="in2chunk")
            chunk_in2[c] = t_
            nc.sync.dma_start(
                t_[:],
                bass.AP(in2_d, c * IN2CH * W,
                        [[HW, 128], [128 * HW, KC], [W, IN2CH], [1, W]]))

        def load_in1_chunk(u):
            t_ = in1_pool.tile([128, KC, IN1CH, W], f32, tag="in1c",
                               name="in1chunk")
            chunk_in1[u] = t_
            nc.sync.dma_start(
                t_[:],
                bass.AP(in1_d, u * IN1CH * W,
                        [[HW, 128], [128 * HW, KC], [W, IN1CH], [1, W]]))

        def pk2_ranges(s, py):
            # (chunk_idx, start_row_in_chunk, r0, nr) covering r in [0,20)
            # natural row of r: 32s - 4 + py + 2r; chunk c = rows [24c, 24c+24)
            if s == 0:
                return [(0, py, 2, 12), (1, py, 14, 6)]
            if s == 1:
                return [(1, 4 + py, 0, 10), (2, py, 10, 10)]
            return [(2, 12 + py, 0, 6), (3, py, 6, 12)]

        def pack_pk1(s):
            pk1_t = {}
            for pi, (py, px) in enumerate(PARITIES):
                pk1_ = pk1_pool.tile([128, KC, NT, 128], f16, tag="pk1",
                                     name="pk1")
                pk1_t[pi] = pk1_
                dap = pk1_[:]
                p_d = dap.ap[0][0]
                for k in range(KC):
                    for ih in range(2):
                        cap = chunk_in1[2 * s + ih][:]
                        src = bass.AP(
                            cap.tensor,
                            cap.offset + k * IN1CH * W + py * W + px,
                            [[cap.ap[0][0], 128], [2 * TW, NT], [2, TW],
                             [2 * W, TH // 2]])
                        dst = bass.AP(
                            dap.tensor,
                            dap.offset + k * NT * 128 + ih * (TH // 2),
                            [[p_d, 128], [128, NT], [TH, TW], [1, TH // 2]])
                        if (pi + k + ih) % 2 == 0:
                            nc.scalar.copy(dst, src)
                        else:
                            nc.vector.tensor_copy(dst, src)
            return pk1_t

        def pack_pk2(s, pi):
            py, px = PARITIES[pi]
            pk2_ = pk2_pool.tile([128, KC, PKC, WH], f16, tag="pk2",
                                 name="pk2")
            dap = pk2_[:]
            p_d = dap.ap[0][0]
            # zero the +-2 parity col halo (s' in [0,2) and [98,100))
            nc.gpsimd.memset(pk2_[:, :, 0:2, :], 0.0)
            nc.gpsimd.memset(pk2_[:, :, PKC - 2:PKC, :], 0.0)
            # zero rows beyond the image (top of strip 0, bottom of last)
            if s == 0:
                nc.gpsimd.memset(pk2_[:, :, :, 0:2], 0.0)
            if s == NSTRIP - 1:
                nc.gpsimd.memset(pk2_[:, :, :, WH - 2:WH], 0.0)
            for k in range(KC):
                for (ci, row0, r0, nr) in pk2_ranges(s, py):
                    cap = chunk_in2[ci][:]
                    src = bass.AP(
                        cap.tensor,
                        cap.offset + k * IN2CH * W + row0 * W + px,
                        [[cap.ap[0][0], 128], [2, PKC - 4], [2 * W, nr]])
                    dst = bass.AP(
                        dap.tensor,
                        dap.offset + k * PKC * WH + 2 * WH + r0,
                        [[p_d, 128], [WH, PKC - 4], [1, nr]])
                    eng = pk2_engine()
                    if eng is nc.scalar:
                        eng.copy(dst, src)
                    else:
                        eng.tensor_copy(dst, src)
            return pk2_

        def matmuls(pk1_, pk2_, S_, pip):
            pk2_ap = pk2_[:]
            p_pk2 = pk2_ap.ap[0][0]
            for t in range(NT):
                pw = pw_pool.tile([128, NW], f32, tag="pw", name="pw")
                for k in range(KC):
                    rhs = bass.AP(pk2_ap.tensor,
                                  pk2_ap.offset + k * PKC * WH + t * TW * WH,
                                  [[p_pk2, 128], [1, NW]])
                    nc.tensor.matmul(pw[:], pk1_[:, k, t, :], rhs,
                                     start=(k == 0), stop=(k == KC - 1))
                if t % 2 == 0:
                    nc.vector.tensor_copy(S_[:, pip, t, :], pw[:])
                else:
                    nc.scalar.copy(S_[:, pip, t, :], pw[:])

        def shear(s, P, S_):
            pair_base = (s % 2) * (4 * RPAR) + 2 * P * RPAR
            s_ap = S_[:]
            p_S = s_ap.ap[0][0]
            for pip in range(2):
                for j in range(TW):
                    sap = bass.AP(s_ap.tensor,
                                  s_ap.offset + 16 * j * p_S + pip * NT * NW
                                  + WH * j,
                                  [[p_S, 16], [NW, NT], [1, NCOLS]])
                    dap = bass.AP(scratch_d,
                                  pair_base + pip * RPAR + 16 * j * PITCH
                                  + SHIFT0,
                                  [[PITCH - 1, 16], [NCOLS, NT], [1, NCOLS]])
                    nc.sync.dma_start(dap, sap)
            U_ = u_pool.tile([128, 2, ULEN], f16, tag="U", name="U")
            nc.sync.dma_start(
                U_[:],
                bass.AP(scratch_d, pair_base + SHIFT0,
                        [[PITCH, 128], [RPAR, 2], [1, ULEN]]))
            return U_

        o_state = {}

        def post(s, P, U_):
            # compact 1185 -> 25, transpose [m,d]->[d,m], assemble to o_sbuf
            if P == 0:
                o_state[s] = o_pool.tile([25, SROWS, W], f32, tag="o",
                                         name="o_sbuf")
            o_sbuf = o_state[s]
            o_ap = o_sbuf[:]
            p_o = o_ap.ap[0][0]
            u_ap = U_[:]
            p_u = u_ap.ap[0][0]
            for pip in range(2):
                pi = 2 * P + pip
                py, px = PARITIES[pi]
                Y_ = y_pool.tile([128, NT, 25], f16, tag="Y", name="Y")
                src = bass.AP(u_ap.tensor, u_ap.offset + pip * ULEN,
                              [[p_u, 128], [NCOLS, NT], [1, 5], [WH, 5]])
                nc.vector.tensor_copy(Y_[:], src)
                p2 = p2_pool.tile([25, NT * 128], f16, tag="p2", name="p2")
                for t in range(NT):
                    nc.tensor.transpose(p2[:, t * 128:(t + 1) * 128],
                                        Y_[:, t, :], identity[:])
                p2_ap = p2[:]
                asrc = bass.AP(p2_ap.tensor, p2_ap.offset,
                               [[p2_ap.ap[0][0], 25], [128, NT], [TH, TW],
                                [1, TH]])
                adst = bass.AP(o_ap.tensor, o_ap.offset + py * W + px,
                               [[p_o, 25], [2 * TW, NT], [2, TW], [2 * W, TH]])
                if pi % 2 == 0:
                    nc.scalar.mul(adst, asrc, inv_c)
                else:
                    nc.vector.tensor_scalar_mul(adst, asrc, inv_c)

        def out_dma(s):
            nc.sync.dma_start(
                bass.AP(out_d, s * SROWS * W, [[HW, 25], [W, SROWS], [1, W]]),
                o_state[s][:])

        # prologue loads
        load_in2_chunk(0)
        load_in2_chunk(1)
        load_in1_chunk(0)
        load_in1_chunk(1)

        U_hist = {}
        for s in range(NSTRIP):
            pk1_t = pack_pk1(s)
            if s > 0:
                post(s - 1, 0, U_hist[(s - 1, 0)])
            if s + 1 < NSTRIP:
                load_in1_chunk(2 * s + 2)
                load_in1_chunk(2 * s + 3)
            for P in range(2):
                S_ = s_pool.tile([128, 2, NT, NW], f16, tag="S", name="S")
                for pip in range(2):
                    pi = 2 * P + pip
                    pk2_ = pack_pk2(s, pi)
                    if pi == 3 and s + 1 < NSTRIP:
                        load_in2_chunk(s + 2)
                    matmuls(pk1_t[pi], pk2_, S_, pip)
                U_hist[(s, P)] = shear(s, P, S_)
                if P == 0 and s > 0:
                    post(s - 1, 1, U_hist[(s - 1, 1)])
                    out_dma(s - 1)
        post(NSTRIP - 1, 0, U_hist[(NSTRIP - 1, 0)])
        post(NSTRIP - 1, 1, U_hist[(NSTRIP - 1, 1)])
        out_dma(NSTRIP - 1)


def scratch_elems():
    return 2 * 4 * 128 * PITCH


def build_module(C=256, H=96, W=192):
    import concourse.bacc as bacc
    import concourse.tile as tile
    nc = bacc.Bacc("TRN2", target_bir_lowering=False, debug=False)
    in1_d = nc.dram_tensor("in1", [C, H, W], f32, kind="ExternalInput")
    in2_d = nc.dram_tensor("in2", [C, H, W], f32, kind="ExternalInput")
    out_d = nc.dram_tensor("out", [25, H, W], f32, kind="ExternalOutput")
    scratch_d = nc.dram_tensor("scratch", [scratch_elems()], f16)
    with tile.TileContext(nc) as tc:
        build_corr(nc, tc, in1_d, in2_d, out_d, scratch_d, C, H, W)
    nc.compile()
    return nc


def reference_np(in1, in2, md=4, st=2):
    in1, in2 = in1[None], in2[None]
    B, C, H, W = in1.shape
    in2p = np.pad(in2, ((0, 0), (0, 0), (md, md), (md, md)))
    outs = []
    for dy in range(0, 2 * md + 1, st):
        for dx in range(0, 2 * md + 1, st):
            outs.append((in1 * in2p[:, :, dy:dy + H, dx:dx + W]).mean(axis=1))
    return np.stack(outs, axis=1)[0]


_NC = None


def _get_nc():
    global _NC
    if _NC is None:
        _NC = build_module(C_FULL, H_FULL, W_FULL)
    return _NC


def kernel(in1, in2):
    from concourse.bass_utils import run_bass_kernel_spmd
    in1 = np.ascontiguousarray(np.asarray(in1, dtype=np.float32))
    in2 = np.ascontiguousarray(np.asarray(in2, dtype=np.float32))
    assert in1.shape == (B_FULL, C_FULL, H_FULL, W_FULL), in1.shape
    nc = _get_nc()
    in_maps = [{"in1": in1[b], "in2": in2[b]} for b in range(B_FULL)]
    res = run_bass_kernel_spmd(nc, in_maps, core_ids=list(range(B_FULL)))
    out = np.stack(
        [np.asarray(res.results[b]["out"]).reshape(25, H_FULL, W_FULL)
         for b in range(B_FULL)], axis=0)
    return out


# revision 4
# speedup vs baseline: 1.1882x; 1.0200x over previous
"""Trainium2 Bass kernel v2: correlation (cost volume) layer.

kernel(in1, in2): full inputs [8, 256, 96, 192] f32 -> output [8, 25, 96, 192] f32.
Sharding: data-parallel over batch, one batch per NeuronCore (8 cores, SPMD).

out[d=(a,b), h, w] = mean_c in1[c,h,w] * in2pad[c, h+2a-4, w+2b-4],  a,b in 0..4

v2 design vs baseline:
- fp16 matmul inputs (1 cyc/row vs fp32's 4; FWL weight-load overlap).
- Tall tiles: TH=16 rows x TW=8 cols on the parity grid -> 8 j-blocks
  (16-partition shear DMAs) instead of 16, 3 strips of 32 rows instead
  of 6 of 16.
- Band extraction via DRAM shear scratch in fp16 with tile-packed
  columns: per (partition, parity) the 12 tiles' 100-col windows abut
  -> one contiguous 2400B descriptor run per partition (full DMA bw),
  and the whole readback is one DMA per parity-pair.
- Shear fused over (tile, parity-pair): 8 shear DMAs + 1 readback per
  pair; ~70 DMAs total vs ~440 (DMA issue costs ~0.6us sequencer each).
- in1/in2 loaded once in chunks (no halo re-read); pk2 packs read
  across chunk boundaries.
- Pack work split across scalar/vector/gpsimd engines, weighted by
  measured per-engine strided-copy rates (vector ~1.4 ns/el, scalar
  ~1.7, gpsimd ~3.5).
- Emission software-pipelined: strip s's post-processing (compact,
  transposes, assemble) is emitted inside strip s+1 so the PE never
  queues dep-blocked transposes ahead of ready matmuls.
"""
import sys
if '/opt/trn_rl_repo' not in sys.path:
    sys.path.insert(0, '/opt/trn_rl_repo')
import numpy as np

import concourse.bass as bass
import concourse.mybir as mybir

f32 = mybir.dt.float32
f16 = mybir.dt.float16

C_FULL, H_FULL, W_FULL = 256, 96, 192
B_FULL = 8

TH, TW = 16, 8            # parity-grid tile: m = 16*j + i
WH, WW = TH + 4, TW + 4   # window 20 rows x 12 col-strips
NW = WH * WW              # 240
BAND = 4 * WH + 4 + 1     # 85 = span of {20b + a}
NCOLS = BAND + TH - 1     # 100 columns written per (row, tile)
SHIFT0 = TH - 1           # 15
PITCH = SHIFT0 + 12 * NCOLS + 1   # 1216 per-partition scratch row (f16 elems)
PARITIES = ((0, 0), (0, 1), (1, 0), (1, 1))  # (py, px); pairs share py
IN2CH = 24                # in2 chunk rows
IN1CH = 16                # in1 chunk rows


def build_corr(nc, tc, in1_d, in2_d, out_d, scratch_d, C, H, W):
    from concourse import masks

    KC = C // 128
    HW = H * W
    NT = (W // 2) // TW       # 12 tiles per parity row-strip
    SROWS = 2 * TH            # 32 natural rows per strip
    NSTRIP = H // SROWS       # 3
    PKC = (W // 2) + 4        # 100 packed plane cols (+-2 parity halo)
    RPAR = 128 * PITCH        # scratch region per parity
    ULEN = (NT - 1) * NCOLS + BAND  # 1185 readback cols per parity
    NCH2 = H // IN2CH + 1     # 4 in2 chunks (last partial-use)
    NCH1 = H // IN1CH         # 6 in1 chunks
    inv_c = 1.0 / C

    with (
        tc.tile_pool(name="const", bufs=1) as cpool,
        tc.tile_pool(name="in2c", bufs=4) as in2_pool,
        tc.tile_pool(name="in1c", bufs=4) as in1_pool,
        tc.tile_pool(name="pk1", bufs=4) as pk1_pool,
        tc.tile_pool(name="pk2", bufs=2) as pk2_pool,
        tc.tile_pool(name="spool", bufs=1) as s_pool,
        tc.tile_pool(name="upool", bufs=2) as u_pool,
        tc.tile_pool(name="ypool", bufs=2) as y_pool,
        tc.tile_pool(name="opool", bufs=1) as o_pool,
        tc.tile_pool(name="psumw", bufs=4, space="PSUM") as pw_pool,
        tc.tile_pool(name="psum2", bufs=2, space="PSUM") as p2_pool,
    ):
        identity = cpool.tile([128, 128], f16)
        masks.make_identity(nc, identity[:])

        # weighted engine cycle for pk2 gathers: capacity-proportional
        PK2_CYCLE = ('g', 'v', 'g', 's', 'g', 'v', 'g', 's')
        pk2_rr = [0]

        def pk2_engine():
            pk2_rr[0] = (pk2_rr[0] + 1) % len(PK2_CYCLE)
            return {'s': nc.scalar, 'v': nc.vector,
                    'g': nc.gpsimd}[PK2_CYCLE[pk2_rr[0]]]

        # per-(chunk, k) tiles so the k=0 packs can start as soon as the
        # k=0 halves land (halves the compute-start latency at prologue)
        chunk_in2 = {}
        chunk_in1 = {}

        def load_in2_chunk(c, k):
            t_ = in2_pool.tile([128, IN2CH, W], f32, tag="in2c",
                               name="in2chunk")
            chunk_in2[(c, k)] = t_
            nc.sync.dma_start(
                t_[:],
                bass.AP(in2_d, k * 128 * HW + c * IN2CH * W,
                        [[HW, 128], [W, IN2CH], [1, W]]))

        def load_in1_chunk(u, k):
            t_ = in1_pool.tile([128, IN1CH, W], f32, tag="in1c",
                               name="in1chunk")
            chunk_in1[(u, k)] = t_
            nc.sync.dma_start(
                t_[:],
                bass.AP(in1_d, k * 128 * HW + u * IN1CH * W,
                        [[HW, 128], [W, IN1CH], [1, W]]))

        def pk2_ranges(s, py):
            # (chunk_idx, start_row_in_chunk, r0, nr) covering r in [0,20)
            # natural row of r: 32s - 4 + py + 2r; chunk c = rows [24c, 24c+24)
            if s == 0:
                return [(0, py, 2, 12), (1, py, 14, 6)]
            if s == 1:
                return [(1, 4 + py, 0, 10), (2, py, 10, 10)]
            return [(2, 12 + py, 0, 6), (3, py, 6, 12)]

        def pack_pk1(s):
            pk1_t = {}
            for pi, (py, px) in enumerate(PARITIES):
                pk1_ = pk1_pool.tile([128, KC, NT, 128], f16, tag="pk1",
                                     name="pk1")
                pk1_t[pi] = pk1_
                dap = pk1_[:]
                p_d = dap.ap[0][0]
                for k in range(KC):
                    for ih in range(2):
                        cap = chunk_in1[(2 * s + ih, k)][:]
                        src = bass.AP(
                            cap.tensor,
                            cap.offset + py * W + px,
                            [[cap.ap[0][0], 128], [2 * TW, NT], [2, TW],
                             [2 * W, TH // 2]])
                        dst = bass.AP(
                            dap.tensor,
                            dap.offset + k * NT * 128 + ih * (TH // 2),
                            [[p_d, 128], [128, NT], [TH, TW], [1, TH // 2]])
                        if (pi + k + ih) % 2 == 0:
                            nc.scalar.copy(dst, src)
                        else:
                            nc.vector.tensor_copy(dst, src)
            return pk1_t

        def pack_pk2(s, pi):
            py, px = PARITIES[pi]
            pk2_ = pk2_pool.tile([128, KC, PKC, WH], f16, tag="pk2",
                                 name="pk2")
            dap = pk2_[:]
            p_d = dap.ap[0][0]
            # zero the +-2 parity col halo (s' in [0,2) and [98,100))
            nc.gpsimd.memset(pk2_[:, :, 0:2, :], 0.0)
            nc.gpsimd.memset(pk2_[:, :, PKC - 2:PKC, :], 0.0)
            # zero rows beyond the image (top of strip 0, bottom of last)
            if s == 0:
                nc.gpsimd.memset(pk2_[:, :, :, 0:2], 0.0)
            if s == NSTRIP - 1:
                nc.gpsimd.memset(pk2_[:, :, :, WH - 2:WH], 0.0)
            for k in range(KC):
                for (ci, row0, r0, nr) in pk2_ranges(s, py):
                    cap = chunk_in2[(ci, k)][:]
                    src = bass.AP(
                        cap.tensor,
                        cap.offset + row0 * W + px,
                        [[cap.ap[0][0], 128], [2, PKC - 4], [2 * W, nr]])
                    dst = bass.AP(
                        dap.tensor,
                        dap.offset + k * PKC * WH + 2 * WH + r0,
                        [[p_d, 128], [WH, PKC - 4], [1, nr]])
                    eng = pk2_engine()
                    if eng is nc.scalar:
                        eng.copy(dst, src)
                    else:
                        eng.tensor_copy(dst, src)
            return pk2_

        def matmuls(pk1_, pk2_, S_, pip):
            # two tiles share one PSUM bank; one 480-el drain copy per pair
            pk2_ap = pk2_[:]
            p_pk2 = pk2_ap.ap[0][0]
            for tp in range(NT // 2):
                pw = pw_pool.tile([128, 2, NW], f32, tag="pw", name="pw")
                for tt in range(2):
                    t = 2 * tp + tt
                    for k in range(KC):
                        rhs = bass.AP(
                            pk2_ap.tensor,
                            pk2_ap.offset + k * PKC * WH + t * TW * WH,
                            [[p_pk2, 128], [1, NW]])
                        nc.tensor.matmul(pw[:, tt, :], pk1_[:, k, t, :], rhs,
                                         start=(k == 0), stop=(k == KC - 1))
                if tp % 2 == 0:
                    nc.vector.tensor_copy(S_[:, pip, 2 * tp:2 * tp + 2, :],
                                          pw[:])
                else:
                    nc.scalar.copy(S_[:, pip, 2 * tp:2 * tp + 2, :], pw[:])

        def shear(s, P, S_):
            pair_base = (s % 2) * (4 * RPAR) + 2 * P * RPAR
            s_ap = S_[:]
            p_S = s_ap.ap[0][0]
            for pip in range(2):
                for j in range(TW):
                    sap = bass.AP(s_ap.tensor,
                                  s_ap.offset + 16 * j * p_S + pip * NT * NW
                                  + WH * j,
                                  [[p_S, 16], [NW, NT], [1, NCOLS]])
                    dap = bass.AP(scratch_d,
                                  pair_base + pip * RPAR + 16 * j * PITCH
                                  + SHIFT0,
                                  [[PITCH - 1, 16], [NCOLS, NT], [1, NCOLS]])
                    nc.sync.dma_start(dap, sap)
            U_ = u_pool.tile([128, 2, ULEN], f16, tag="U", name="U")
            nc.sync.dma_start(
                U_[:],
                bass.AP(scratch_d, pair_base + SHIFT0,
                        [[PITCH, 128], [RPAR, 2], [1, ULEN]]))
            return U_

        o_state = {}

        def post(s, P, U_):
            # compact 1185 -> 25, transpose [m,d]->[d,m], assemble to o_sbuf
            if P == 0:
                o_state[s] = o_pool.tile([25, SROWS, W], f32, tag="o",
                                         name="o_sbuf")
            o_sbuf = o_state[s]
            o_ap = o_sbuf[:]
            p_o = o_ap.ap[0][0]
            u_ap = U_[:]
            p_u = u_ap.ap[0][0]
            for pip in range(2):
                pi = 2 * P + pip
                py, px = PARITIES[pi]
                Y_ = y_pool.tile([128, NT, 25], f16, tag="Y", name="Y")
                src = bass.AP(u_ap.tensor, u_ap.offset + pip * ULEN,
                              [[p_u, 128], [NCOLS, NT], [1, 5], [WH, 5]])
                nc.vector.tensor_copy(Y_[:], src)
                p2 = p2_pool.tile([25, NT * 128], f16, tag="p2", name="p2")
                for t in range(NT):
                    nc.tensor.transpose(p2[:, t * 128:(t + 1) * 128],
                                        Y_[:, t, :], identity[:])
                p2_ap = p2[:]
                asrc = bass.AP(p2_ap.tensor, p2_ap.offset,
                               [[p2_ap.ap[0][0], 25], [128, NT], [TH, TW],
                                [1, TH]])
                adst = bass.AP(o_ap.tensor, o_ap.offset + py * W + px,
                               [[p_o, 25], [2 * TW, NT], [2, TW], [2 * W, TH]])
                if pi % 2 == 0:
                    nc.scalar.mul(adst, asrc, inv_c)
                else:
                    nc.vector.tensor_scalar_mul(adst, asrc, inv_c)

        def out_dma(s):
            nc.sync.dma_start(
                bass.AP(out_d, s * SROWS * W, [[HW, 25], [W, SROWS], [1, W]]),
                o_state[s][:])

        # prologue loads: k=0 halves first so the first packs/matmuls can
        # start before the k=1 halves arrive
        for k in range(KC):
            load_in2_chunk(0, k)
            load_in2_chunk(1, k)
            load_in1_chunk(0, k)
            load_in1_chunk(1, k)

        U_hist = {}
        for s in range(NSTRIP):
            pk1_t = pack_pk1(s)
            if s > 0:
                post(s - 1, 0, U_hist[(s - 1, 0)])
            if s + 1 < NSTRIP:
                for k in range(KC):
                    load_in1_chunk(2 * s + 2, k)
                    load_in1_chunk(2 * s + 3, k)
            for P in range(2):
                S_ = s_pool.tile([128, 2, NT, NW], f16, tag="S", name="S")
                for pip in range(2):
                    pi = 2 * P + pip
                    pk2_ = pack_pk2(s, pi)
                    if pi == 3 and s + 1 < NSTRIP:
                        load_in2_chunk(s + 2, 0)
                        load_in2_chunk(s + 2, 1)
                    matmuls(pk1_t[pi], pk2_, S_, pip)
                U_hist[(s, P)] = shear(s, P, S_)
                if P == 0 and s > 0:
                    post(s - 1, 1, U_hist[(s - 1, 1)])
                    out_dma(s - 1)
                if P == 0 and s == NSTRIP - 1:
                    # tail shrink: last strip's first post overlaps pair 1
                    post(s, 0, U_hist[(s, 0)])
        post(NSTRIP - 1, 1, U_hist[(NSTRIP - 1, 1)])
        out_dma(NSTRIP - 1)


def scratch_elems():
    return 2 * 4 * 128 * PITCH


def build_module(C=256, H=96, W=192):
    import concourse.bacc as bacc
    import concourse.tile as tile
    nc = bacc.Bacc("TRN2", target_bir_lowering=False, debug=False)
    in1_d = nc.dram_tensor("in1", [C, H, W], f32, kind="ExternalInput")
    in2_d = nc.dram_tensor("in2", [C, H, W], f32, kind="ExternalInput")
    out_d = nc.dram_tensor("out", [25, H, W], f32, kind="ExternalOutput")
    scratch_d = nc.dram_tensor("scratch", [scratch_elems()], f16)
    with tile.TileContext(nc) as tc:
        build_corr(nc, tc, in1_d, in2_d, out_d, scratch_d, C, H, W)
    nc.compile()
    return nc


def reference_np(in1, in2, md=4, st=2):
    in1, in2 = in1[None], in2[None]
    B, C, H, W = in1.shape
    in2p = np.pad(in2, ((0, 0), (0, 0), (md, md), (md, md)))
    outs = []
    for dy in range(0, 2 * md + 1, st):
        for dx in range(0, 2 * md + 1, st):
            outs.append((in1 * in2p[:, :, dy:dy + H, dx:dx + W]).mean(axis=1))
    return np.stack(outs, axis=1)[0]


_NC = None


def _get_nc():
    global _NC
    if _NC is None:
        _NC = build_module(C_FULL, H_FULL, W_FULL)
    return _NC


def kernel(in1, in2):
    from concourse.bass_utils import run_bass_kernel_spmd
    in1 = np.ascontiguousarray(np.asarray(in1, dtype=np.float32))
    in2 = np.ascontiguousarray(np.asarray(in2, dtype=np.float32))
    assert in1.shape == (B_FULL, C_FULL, H_FULL, W_FULL), in1.shape
    nc = _get_nc()
    in_maps = [{"in1": in1[b], "in2": in2[b]} for b in range(B_FULL)]
    res = run_bass_kernel_spmd(nc, in_maps, core_ids=list(range(B_FULL)))
    out = np.stack(
        [np.asarray(res.results[b]["out"]).reshape(25, H_FULL, W_FULL)
         for b in range(B_FULL)], axis=0)
    return out
